# revision 48
# baseline (speedup 1.0000x reference)
"""Causal self-attention (B=2, T=2048, C=1024, H=16, D=64) on 8 Trainium2
NeuronCores.

Sharding: core = (batch, head-group): b = core // 4, hg = core % 4; each core
computes 4 heads of one batch plus its partial out-projection (256 of 1024
contraction channels). Host sums the 4 partial y's per batch.

Per-core pipeline (all matmuls float32r: 1 cycle/row at N=512, fp32 accumulate):
  1. qT/kT = (w_qk.T).T @ xT  -> [head-pair*64, T] layout (D on partitions)
     v     = xT.T @ w_v.T     -> [T, 4*65] with a ones column per head
  2. scores^T[tk, tq] = kT.T @ qT per head, block [128, 512], causal block-skip
     exp on ACT (PSUM->SBUF, f32r out); attention scale 1/8 pre-folded into w_q
     causal mask via gpsimd affine_select on diagonal blocks (post-exp, fill 0)
  3. outT[65, tq] = [v|1].T @ expS^T accumulated over tk tiles; row 64 is the
     softmax denominator for free. Normalize: reciprocal + partition_broadcast
     + tensor_mul into pair-stacked attnout^T tiles [128, 512] (f32r).
  4. y[t, o] += attnout^T.T @ w_out^T per head-pair (K=128), PSUM->SBUF->DRAM.
"""

import numpy as np

B, T, C = 2, 2048, 1024
H, D = 16, 64
N_CORES = 8
HG = 4  # head-groups (cores per batch)
HPC = 4  # heads per core
NCH = T // 512  # 4 tq chunks of 512
KT = T // 128  # 16 tk tiles of 128
CK = C // 128  # 8 contraction k-tiles for the projections

_NC = None


def _build_nc():
    import concourse.mybir as mybir
    import concourse.tile as tile
    from concourse import bacc

    F32 = mybir.dt.float32
    F32R = mybir.dt.float32r

    nc = bacc.Bacc(None, target_bir_lowering=False)
    xT = nc.dram_tensor("xT", [128, CK, T], F32R, kind="ExternalInput")
    wqk = nc.dram_tensor("wqk", [128, CK, 512], F32R, kind="ExternalInput")
    wv = nc.dram_tensor("wv", [128, CK, 256], F32R, kind="ExternalInput")
    wout = nc.dram_tensor("wout", [2, 128, C], F32R, kind="ExternalInput")
    ones = nc.dram_tensor("ones", [128, 4], F32R, kind="ExternalInput")
    y = nc.dram_tensor("y", [T, C], F32, kind="ExternalOutput")

    with tile.TileContext(nc) as tc:
        with (
            tc.tile_pool(name="const", bufs=1) as const,
            tc.tile_pool(name="xin", bufs=3) as xin,
            tc.tile_pool(name="qk", bufs=1) as qkp,
            tc.tile_pool(name="vt", bufs=1) as vtp,
            tc.tile_pool(name="es", bufs=9) as esp,
            tc.tile_pool(name="ao", bufs=1) as aop,
            tc.tile_pool(name="nrm", bufs=3) as nrm,
            tc.tile_pool(name="yo", bufs=3) as yop,
            tc.tile_pool(name="mm", bufs=2, space="PSUM") as mmps,
            tc.tile_pool(name="sc", bufs=2, space="PSUM") as scps,
            tc.tile_pool(name="av", bufs=2, space="PSUM") as avps,
        ):
            # ---- weights + first x chunk, DMA-interleaved so the first
            # proj matmul (needs wqk_0 + xt0_0) can issue ~2 us in ----
            wqk_t = []
            xt_first = []
            for k in range(4):
                t_ = const.tile([128, 512], F32R, tag=f"wqk{k}")
                nc.sync.dma_start(t_[:], wqk[:, k, :])
                wqk_t.append(t_)
                t_ = xin.tile([128, 512], F32R, tag=f"xt{k}", name=f"xt0_{k}", bufs=1)
                nc.sync.dma_start(t_[:], xT[:, k, 0:512])
                xt_first.append(t_)
            wqk_hi = const.tile([128, 4 * 512], F32R, tag="wqkhi")
            nc.sync.dma_start(
                wqk_hi[:].rearrange("p (g c) -> p g c", g=4), wqk[:, 4:8, :]
            )
            wqk_t += [wqk_hi[:, k * 512 : (k + 1) * 512] for k in range(4)]
            for k in range(4, CK):
                t_ = xin.tile([128, 512], F32R, tag=f"xt{k}", name=f"xt0_{k}", bufs=1)
                nc.sync.dma_start(t_[:], xT[:, k, 0:512])
                xt_first.append(t_)
            ones_sb = const.tile([128, 4], F32R, tag="ones")
            nc.sync.dma_start(ones_sb[:], ones[:])
            wv_all = const.tile([128, CK * 256], F32R, tag="wvall")
            nc.sync.dma_start(
                wv_all[:].rearrange("p (g c) -> p g c", g=CK), wv[:, :, :]
            )
            wv_t = [wv_all[:, k * 256 : (k + 1) * 256] for k in range(CK)]

            # ---- v tiles with ones columns ----
            v_t = []
            for t in range(KT):
                t_ = vtp.tile([128, 4 * 65], F32R, tag=f"v{t}")
                nc.vector.tensor_copy(
                    t_[:].rearrange("p (h e) -> p h e", h=4)[:, :, 64:65],
                    ones_sb[:].rearrange("p (h e) -> p h e", e=1),
                )
                v_t.append(t_)

            # ---- projections, streamed by tq/n chunk ----
            # qkT[m] tiles per (m, n): m 0..1 = q head-pairs, 2..3 = k pairs
            qkT = [[None] * NCH for _ in range(4)]

            def proj_chunk(n):
                if n == 0:
                    xt_n = xt_first
                else:
                    xt_n = []
                    for half in range(2):
                        big = xin.tile(
                            [128, 4 * 512], F32R, tag=f"xth{half}",
                            name=f"xth{half}_{n}", bufs=2,
                        )
                        nc.sync.dma_start(
                            big[:].rearrange("p (g t) -> p g t", g=4),
                            xT[:, half * 4 : half * 4 + 4, n * 512 : (n + 1) * 512],
                        )
                        xt_n += [big[:, k * 512 : (k + 1) * 512] for k in range(4)]
                for m in range(4):
                    ps = mmps.tile([128, 512], F32, tag="mm")
                    for k in range(CK):
                        nc.tensor.matmul(
                            ps[:],
                            wqk_t[k][:, m * 128 : (m + 1) * 128],
                            xt_n[k][:],
                            start=(k == 0),
                            stop=(k == CK - 1),
                        )
                    sb = qkp.tile([128, 512], F32R, tag=f"qk{m}_{n}", name=f"qk{m}_{n}")
                    nc.vector.tensor_copy(sb[:], ps[:])
                    qkT[m][n] = sb
                for ts in range(4):  # t-subtiles of this chunk
                    t = n * 4 + ts
                    ps = mmps.tile([128, 256], F32, tag="mm")
                    for k in range(CK):
                        nc.tensor.matmul(
                            ps[:],
                            xt_n[k][:, ts * 128 : (ts + 1) * 128],
                            wv_t[k][:],
                            start=(k == 0),
                            stop=(k == CK - 1),
                        )
                    nc.vector.tensor_copy(
                        v_t[t][:].rearrange("p (h e) -> p h e", h=4)[:, :, 0:64],
                        ps[:].rearrange("p (h d) -> p h d", h=4),
                    )

            # ---- attention + out-projection, per tq chunk ----
            # Blocks are software-pipelined: scores+exp run one block ahead of
            # the attn@v accumulation so PE never stalls on ACT. Off-diagonal
            # tk tiles are computed in pairs sharing one [128,1024] exp; the 4
            # diagonal tiles get trimmed exps (only the column range that can
            # be valid) and width-trimmed affine_select causal masks.
            attnout = [[None] * NCH for _ in range(2)]
            wout_t = []
            for p in range(2):
                t_ = const.tile([128, C], F32R, tag=f"wout{p}")
                nc.sync.dma_start(t_[:], wout[p])
                wout_t.append(t_)

            def emit_unit_blocks(h, q, pav):
                """Yields av-emit thunks, one per block, after emitting that
                block's scores+exp instructions."""
                p, s = h // 2, h % 2
                r0, r1 = 64 * s, 64 * s + 64
                vslice = slice(65 * h, 65 * h + 65)
                last_j = 4 * q + 3

                def sc_mm(out_ap, j, trim=0):
                    nc.tensor.matmul(
                        out_ap,
                        qkT[2 + p][j // 4][r0:r1, (j % 4) * 128 : (j % 4 + 1) * 128],
                        qkT[p][q][r0:r1, trim:512],
                        start=True,
                        stop=True,
                    )

                def av_mm(pav, es_ap, j, trim=0):
                    nc.tensor.matmul(
                        pav[:, trim:512],
                        v_t[j][:, vslice],
                        es_ap,
                        start=(j == 0),
                        stop=(j == last_j),
                    )

                for j0 in range(0, 4 * q, 2):  # off-diagonal pairs
                    psc = scps.tile([128, 1024], F32, tag="sc")
                    sc_mm(psc[:, 0:512], j0)
                    sc_mm(psc[:, 512:1024], j0 + 1)
                    es = esp.tile([128, 1024], F32R, tag="es")
                    nc.scalar.activation(
                        es[:], psc[:], mybir.ActivationFunctionType.Exp
                    )

                    def av(es=es, j0=j0):
                        av_mm(pav, es[:, 0:512], j0)
                        av_mm(pav, es[:, 512:1024], j0 + 1)

                    yield av
                for j in range(4 * q, 4 * q + 4):  # diagonal tiles
                    d = j - 4 * q
                    off = 128 * d
                    # scores/attn@v only need columns that can be valid, but
                    # f32r needs moving size >= 256 for full rate: trim to
                    # [128d:512] for d=1,2 and keep full width for d=0,3.
                    trim = off if d in (1, 2) else 0
                    psc = mmps.tile([128, 512], F32, tag="mm", name="pscd")
                    sc_mm(psc[:, trim:512], j, trim)
                    es = esp.tile([128, 1024], F32R, tag="es")
                    nc.scalar.activation(
                        es[:, off:512],
                        psc[:, off:512],
                        mybir.ActivationFunctionType.Exp,
                    )
                    # keep tq - tk >= 0: f - part - 128*d >= 0. Only columns
                    # f < 128*(d+1) can be invalid; garbage below the exp
                    # window is entirely in the filled region.
                    nc.gpsimd.affine_select(
                        out=es[:, 0 : 128 * (d + 1)],
                        in_=es[:, 0 : 128 * (d + 1)],
                        compare_op=mybir.AluOpType.is_ge,
                        fill=0.0,
                        base=-off,
                        pattern=[[1, 128 * (d + 1)]],
                        channel_multiplier=-1,
                    )

                    def av(es=es, j=j, trim=trim):
                        av_mm(pav, es[:, trim:512], j, trim)

                    yield av

            def normalize(h, q, pav):
                p, s = h // 2, h % 2
                r0, r1 = 64 * s, 64 * s + 64
                rcp = nrm.tile([1, 512], F32, tag="rcp")
                nc.vector.reciprocal(rcp[:], pav[64:65, :])
                rb = nrm.tile([64, 512], F32, tag="rb")
                nc.gpsimd.partition_broadcast(rb[:], rcp[0:1, :])
                nc.vector.tensor_mul(attnout[p][q][r0:r1, :], pav[0:64, :], rb[:])

            ysb_open = {}

            def outproj_group(q, ts, oc, use_act=False):
                t = q * 4 + ts
                py = mmps.tile([128, 512], F32, tag="mm")
                for p in range(2):
                    nc.tensor.matmul(
                        py[:],
                        attnout[p][q][:, ts * 128 : (ts + 1) * 128],
                        wout_t[p][:, oc * 512 : (oc + 1) * 512],
                        start=(p == 0),
                        stop=(p == 1),
                    )
                if oc == 0:
                    ysb = yop.tile([128, 1024], F32, tag="y", name=f"y{q}_{ts}")
                    ysb_open[(q, ts)] = ysb
                else:
                    ysb = ysb_open.pop((q, ts))
                if use_act:
                    nc.scalar.copy(ysb[:, oc * 512 : (oc + 1) * 512], py[:])
                else:
                    nc.vector.tensor_copy(ysb[:, oc * 512 : (oc + 1) * 512], py[:])
                if oc == 1:
                    nc.sync.dma_start(y[t * 128 : (t + 1) * 128, :], ysb[:])

            # Flat emission, load-levelled: projection chunks n>=2 interleave
            # with early attention chunks (their ACT load is light, so the
            # proj matmuls fill PE while ACT churns); out-projection groups
            # are sprinkled between units as PE filler for the ACT-paced
            # late chunks. A chunk's outproj groups become eligible only
            # once its last unit's normalize has been EMITTED (Tile derives
            # dependencies from program order).
            pending = None  # (av_thunk, normalize_thunk, after_thunks)
            pending_outproj = []  # eligible outproj group thunks

            def flush_pending():
                nonlocal pending
                if pending is not None:
                    pending[0]()
                    pending[1]()
                    pending_outproj.extend(pending[2])
                    pending = None

            def attn_chunk(q):
                nonlocal pending
                for p in range(2):
                    attnout[p][q] = aop.tile(
                        [128, 512], F32R, tag=f"ao{p}_{q}", name=f"ao{p}_{q}"
                    )
                for h in range(HPC):
                    pav = avps.tile([65, 512], F32, tag="av")
                    prev_av = None
                    for av in emit_unit_blocks(h, q, pav):
                        if prev_av is not None:
                            prev_av()
                        elif pending is not None:
                            flush_pending()
                        prev_av = av
                    after = (
                        [
                            (
                                lambda q=q, ts=ts, oc=oc: outproj_group(
                                    q, ts, oc, use_act=(q == NCH - 1 and oc == 0)
                                )
                            )
                            for ts in range(4)
                            for oc in range(2)
                        ]
                        if h == HPC - 1
                        else []
                    )
                    pending = (
                        prev_av,
                        lambda h=h, q=q, pav=pav: normalize(h, q, pav),
                        after,
                    )
                    for _ in range(2):
                        if pending_outproj:
                            pending_outproj.pop(0)()

            proj_chunk(0)
            proj_chunk(1)
            attn_chunk(0)
            proj_chunk(2)
            attn_chunk(1)
            proj_chunk(3)
            attn_chunk(2)
            attn_chunk(3)
            flush_pending()
            for th in pending_outproj:
                th()

    nc.finalize()
    return nc


def _prep_core_inputs(x, w_qkv, w_out, core):
    b, hg = core // HG, core % HG
    xT = np.ascontiguousarray(x[b].T)
    wq = w_qkv[0:C] * np.float32(1.0 / np.sqrt(D))
    wk = w_qkv[C : 2 * C]
    wv = w_qkv[2 * C : 3 * C]
    h0 = HPC * hg
    rows = []
    for p in range(2):
        rows.append(wq[64 * (h0 + 2 * p) : 64 * (h0 + 2 * p + 2)])
    for p in range(2):
        rows.append(wk[64 * (h0 + 2 * p) : 64 * (h0 + 2 * p + 2)])
    wqk_lhsT = np.ascontiguousarray(
        np.concatenate(rows, axis=0).T.reshape(CK, 128, 512).transpose(1, 0, 2)
    )
    wv_rhsT = np.ascontiguousarray(wv[64 * h0 : 64 * (h0 + HPC)].T)
    wout_pairs = np.ascontiguousarray(
        w_out[:, 64 * h0 : 64 * (h0 + HPC)].T
    ).reshape(2, 128, C)
    # xT and wv are sent p-major shuffled ([128, CK, ...]) so the kernel can
    # load several contraction k-tiles with one contiguous DMA.
    xTs = np.ascontiguousarray(
        xT.reshape(CK, 128, T).transpose(1, 0, 2)
    )
    wvs = np.ascontiguousarray(
        wv_rhsT.reshape(CK, 128, 256).transpose(1, 0, 2)
    )
    return {
        "ones": np.ones((128, 4), dtype=np.float32),
        "xT": xTs.astype(np.float32),
        "wqk": wqk_lhsT.astype(np.float32),
        "wv": wvs.astype(np.float32),
        "wout": wout_pairs.astype(np.float32),
    }


def kernel(x, w_qkv, w_out):
    from concourse.bass_utils import run_bass_kernel_spmd

    global _NC
    x = np.asarray(x, dtype=np.float32)
    w_qkv = np.asarray(w_qkv, dtype=np.float32)
    w_out = np.asarray(w_out, dtype=np.float32)

    in_maps = [_prep_core_inputs(x, w_qkv, w_out, c) for c in range(N_CORES)]
    if _NC is None:
        _NC = _build_nc()
    res = run_bass_kernel_spmd(_NC, in_maps, core_ids=list(range(N_CORES)))
    out = np.zeros((B, T, C), dtype=np.float32)
    for c in range(N_CORES):
        out[c // HG] += res.results[c]["y"]
    return out


# revision 51
# speedup vs baseline: 1.0010x; 1.0010x over previous
"""Causal self-attention (B=2, T=2048, C=1024, H=16, D=64) on 8 Trainium2
NeuronCores.

Sharding: core = (batch, head-group): b = core // 4, hg = core % 4; each core
computes 4 heads of one batch plus its partial out-projection (256 of 1024
contraction channels). Host sums the 4 partial y's per batch.

Per-core pipeline (all matmuls float32r: 1 cycle/row at N=512, fp32 accumulate):
  1. qT/kT = (w_qk.T).T @ xT  -> [head-pair*64, T] layout (D on partitions)
     v     = xT.T @ w_v.T     -> [T, 4*65] with a ones column per head
  2. scores^T[tk, tq] = kT.T @ qT per head, block [128, 512], causal block-skip
     exp on ACT (PSUM->SBUF, f32r out); attention scale 1/8 pre-folded into w_q
     causal mask via gpsimd affine_select on diagonal blocks (post-exp, fill 0)
  3. outT[65, tq] = [v|1].T @ expS^T accumulated over tk tiles; row 64 is the
     softmax denominator for free. Normalize: reciprocal + partition_broadcast
     + tensor_mul into pair-stacked attnout^T tiles [128, 512] (f32r).
  4. y[t, o] += attnout^T.T @ w_out^T per head-pair (K=128), PSUM->SBUF->DRAM.
"""

import numpy as np

B, T, C = 2, 2048, 1024
H, D = 16, 64
N_CORES = 8
HG = 4  # head-groups (cores per batch)
HPC = 4  # heads per core
NCH = T // 512  # 4 tq chunks of 512
KT = T // 128  # 16 tk tiles of 128
CK = C // 128  # 8 contraction k-tiles for the projections

_NC = None


def _build_nc():
    import concourse.mybir as mybir
    import concourse.tile as tile
    from concourse import bacc

    F32 = mybir.dt.float32
    F32R = mybir.dt.float32r

    nc = bacc.Bacc(None, target_bir_lowering=False)
    xT = nc.dram_tensor("xT", [128, CK, T], F32R, kind="ExternalInput")
    wqk = nc.dram_tensor("wqk", [128, CK, 512], F32R, kind="ExternalInput")
    wv = nc.dram_tensor("wv", [128, CK, 256], F32R, kind="ExternalInput")
    wout = nc.dram_tensor("wout", [2, 128, C], F32R, kind="ExternalInput")
    ones = nc.dram_tensor("ones", [128, 4], F32R, kind="ExternalInput")
    y = nc.dram_tensor("y", [T, C], F32, kind="ExternalOutput")

    with tile.TileContext(nc) as tc:
        with (
            tc.tile_pool(name="const", bufs=1) as const,
            tc.tile_pool(name="xin", bufs=3) as xin,
            tc.tile_pool(name="qk", bufs=1) as qkp,
            tc.tile_pool(name="vt", bufs=1) as vtp,
            tc.tile_pool(name="es", bufs=8) as esp,
            tc.tile_pool(name="ao", bufs=1) as aop,
            tc.tile_pool(name="nrm", bufs=3) as nrm,
            tc.tile_pool(name="yo", bufs=4) as yop,
            tc.tile_pool(name="mm", bufs=2, space="PSUM") as mmps,
            tc.tile_pool(name="sc", bufs=2, space="PSUM") as scps,
            tc.tile_pool(name="av", bufs=2, space="PSUM") as avps,
        ):
            # ---- weights + first x chunk, DMA-interleaved so the first
            # proj matmul (needs wqk_0 + xt0_0) can issue ~2 us in ----
            wqk_t = []
            xt_first = []
            for k in range(4):
                t_ = const.tile([128, 512], F32R, tag=f"wqk{k}")
                nc.sync.dma_start(t_[:], wqk[:, k, :])
                wqk_t.append(t_)
                t_ = xin.tile([128, 512], F32R, tag=f"xt{k}", name=f"xt0_{k}", bufs=1)
                nc.sync.dma_start(t_[:], xT[:, k, 0:512])
                xt_first.append(t_)
            wqk_hi = const.tile([128, 4 * 512], F32R, tag="wqkhi")
            nc.sync.dma_start(
                wqk_hi[:].rearrange("p (g c) -> p g c", g=4), wqk[:, 4:8, :]
            )
            wqk_t += [wqk_hi[:, k * 512 : (k + 1) * 512] for k in range(4)]
            for k in range(4, CK):
                t_ = xin.tile([128, 512], F32R, tag=f"xt{k}", name=f"xt0_{k}", bufs=1)
                nc.sync.dma_start(t_[:], xT[:, k, 0:512])
                xt_first.append(t_)
            ones_sb = const.tile([128, 4], F32R, tag="ones")
            nc.sync.dma_start(ones_sb[:], ones[:])
            wv_all = const.tile([128, CK * 256], F32R, tag="wvall")
            nc.sync.dma_start(
                wv_all[:].rearrange("p (g c) -> p g c", g=CK), wv[:, :, :]
            )
            wv_t = [wv_all[:, k * 256 : (k + 1) * 256] for k in range(CK)]

            # ---- v tiles with ones columns ----
            v_t = []
            for t in range(KT):
                t_ = vtp.tile([128, 4 * 65], F32R, tag=f"v{t}")
                nc.vector.tensor_copy(
                    t_[:].rearrange("p (h e) -> p h e", h=4)[:, :, 64:65],
                    ones_sb[:].rearrange("p (h e) -> p h e", e=1),
                )
                v_t.append(t_)

            # ---- projections, streamed by tq/n chunk ----
            # qkT[m] tiles per (m, n): m 0..1 = q head-pairs, 2..3 = k pairs
            qkT = [[None] * NCH for _ in range(4)]

            def proj_chunk(n):
                if n == 0:
                    xt_n = xt_first
                else:
                    xt_n = []
                    for half in range(2):
                        big = xin.tile(
                            [128, 4 * 512], F32R, tag=f"xth{half}",
                            name=f"xth{half}_{n}", bufs=2,
                        )
                        nc.sync.dma_start(
                            big[:].rearrange("p (g t) -> p g t", g=4),
                            xT[:, half * 4 : half * 4 + 4, n * 512 : (n + 1) * 512],
                        )
                        xt_n += [big[:, k * 512 : (k + 1) * 512] for k in range(4)]
                for m in range(4):
                    ps = mmps.tile([128, 512], F32, tag="mm")
                    for k in range(CK):
                        nc.tensor.matmul(
                            ps[:],
                            wqk_t[k][:, m * 128 : (m + 1) * 128],
                            xt_n[k][:],
                            start=(k == 0),
                            stop=(k == CK - 1),
                        )
                    sb = qkp.tile([128, 512], F32R, tag=f"qk{m}_{n}", name=f"qk{m}_{n}")
                    nc.vector.tensor_copy(sb[:], ps[:])
                    qkT[m][n] = sb
                for ts in range(4):  # t-subtiles of this chunk
                    t = n * 4 + ts
                    ps = mmps.tile([128, 256], F32, tag="mm")
                    for k in range(CK):
                        nc.tensor.matmul(
                            ps[:],
                            xt_n[k][:, ts * 128 : (ts + 1) * 128],
                            wv_t[k][:],
                            start=(k == 0),
                            stop=(k == CK - 1),
                        )
                    nc.vector.tensor_copy(
                        v_t[t][:].rearrange("p (h e) -> p h e", h=4)[:, :, 0:64],
                        ps[:].rearrange("p (h d) -> p h d", h=4),
                    )

            # ---- attention + out-projection, per tq chunk ----
            # Blocks are software-pipelined: scores+exp run one block ahead of
            # the attn@v accumulation so PE never stalls on ACT. Off-diagonal
            # tk tiles are computed in pairs sharing one [128,1024] exp; the 4
            # diagonal tiles get trimmed exps (only the column range that can
            # be valid) and width-trimmed affine_select causal masks.
            attnout = [[None] * NCH for _ in range(2)]
            wout_t = []
            for p in range(2):
                t_ = const.tile([128, C], F32R, tag=f"wout{p}")
                nc.sync.dma_start(t_[:], wout[p])
                wout_t.append(t_)

            def emit_unit_blocks(h, q, pav):
                """Yields av-emit thunks, one per block, after emitting that
                block's scores+exp instructions."""
                p, s = h // 2, h % 2
                r0, r1 = 64 * s, 64 * s + 64
                vslice = slice(65 * h, 65 * h + 65)
                last_j = 4 * q + 3

                def sc_mm(out_ap, j, trim=0):
                    nc.tensor.matmul(
                        out_ap,
                        qkT[2 + p][j // 4][r0:r1, (j % 4) * 128 : (j % 4 + 1) * 128],
                        qkT[p][q][r0:r1, trim:512],
                        start=True,
                        stop=True,
                    )

                def av_mm(pav, es_ap, j, trim=0):
                    nc.tensor.matmul(
                        pav[:, trim:512],
                        v_t[j][:, vslice],
                        es_ap,
                        start=(j == 0),
                        stop=(j == last_j),
                    )

                for j0 in range(0, 4 * q, 2):  # off-diagonal pairs
                    psc = scps.tile([128, 1024], F32, tag="sc")
                    sc_mm(psc[:, 0:512], j0)
                    sc_mm(psc[:, 512:1024], j0 + 1)
                    es = esp.tile([128, 1024], F32R, tag="es")
                    nc.scalar.activation(
                        es[:], psc[:], mybir.ActivationFunctionType.Exp
                    )

                    def av(es=es, j0=j0):
                        av_mm(pav, es[:, 0:512], j0)
                        av_mm(pav, es[:, 512:1024], j0 + 1)

                    yield av
                for j in range(4 * q, 4 * q + 4):  # diagonal tiles
                    d = j - 4 * q
                    off = 128 * d
                    # scores/attn@v only need columns that can be valid, but
                    # f32r needs moving size >= 256 for full rate: trim to
                    # [128d:512] for d=1,2 and keep full width for d=0,3.
                    trim = off if d in (1, 2) else 0
                    psc = mmps.tile([128, 512], F32, tag="mm", name="pscd")
                    sc_mm(psc[:, trim:512], j, trim)
                    es = esp.tile([128, 1024], F32R, tag="es")
                    nc.scalar.activation(
                        es[:, off:512],
                        psc[:, off:512],
                        mybir.ActivationFunctionType.Exp,
                    )
                    # keep tq - tk >= 0: f - part - 128*d >= 0. Only columns
                    # f < 128*(d+1) can be invalid; garbage below the exp
                    # window is entirely in the filled region.
                    nc.gpsimd.affine_select(
                        out=es[:, 0 : 128 * (d + 1)],
                        in_=es[:, 0 : 128 * (d + 1)],
                        compare_op=mybir.AluOpType.is_ge,
                        fill=0.0,
                        base=-off,
                        pattern=[[1, 128 * (d + 1)]],
                        channel_multiplier=-1,
                    )

                    def av(es=es, j=j, trim=trim):
                        av_mm(pav, es[:, trim:512], j, trim)

                    yield av

            def normalize(h, q, pav):
                p, s = h // 2, h % 2
                r0, r1 = 64 * s, 64 * s + 64
                rcp = nrm.tile([1, 512], F32, tag="rcp")
                nc.vector.reciprocal(rcp[:], pav[64:65, :])
                rb = nrm.tile([64, 512], F32, tag="rb")
                nc.gpsimd.partition_broadcast(rb[:], rcp[0:1, :])
                nc.vector.tensor_mul(attnout[p][q][r0:r1, :], pav[0:64, :], rb[:])

            ysb_open = {}

            def outproj_group(q, ts, oc, use_act=False):
                t = q * 4 + ts
                py = mmps.tile([128, 512], F32, tag="mm")
                for p in range(2):
                    nc.tensor.matmul(
                        py[:],
                        attnout[p][q][:, ts * 128 : (ts + 1) * 128],
                        wout_t[p][:, oc * 512 : (oc + 1) * 512],
                        start=(p == 0),
                        stop=(p == 1),
                    )
                if oc == 0:
                    ysb = yop.tile([128, 1024], F32, tag="y", name=f"y{q}_{ts}")
                    ysb_open[(q, ts)] = ysb
                else:
                    ysb = ysb_open.pop((q, ts))
                if use_act:
                    nc.scalar.copy(ysb[:, oc * 512 : (oc + 1) * 512], py[:])
                else:
                    nc.vector.tensor_copy(ysb[:, oc * 512 : (oc + 1) * 512], py[:])
                if oc == 1:
                    nc.sync.dma_start(y[t * 128 : (t + 1) * 128, :], ysb[:])

            # Flat emission, load-levelled: projection chunks n>=2 interleave
            # with early attention chunks (their ACT load is light, so the
            # proj matmuls fill PE while ACT churns); out-projection groups
            # are sprinkled between units as PE filler for the ACT-paced
            # late chunks. A chunk's outproj groups become eligible only
            # once its last unit's normalize has been EMITTED (Tile derives
            # dependencies from program order).
            pending = None  # (av_thunk, normalize_thunk, after_thunks)
            pending_outproj = []  # eligible outproj group thunks

            def flush_pending():
                nonlocal pending
                if pending is not None:
                    pending[0]()
                    pending[1]()
                    pending_outproj.extend(pending[2])
                    pending = None

            def attn_chunk(q):
                nonlocal pending
                for p in range(2):
                    attnout[p][q] = aop.tile(
                        [128, 512], F32R, tag=f"ao{p}_{q}", name=f"ao{p}_{q}"
                    )
                for h in range(HPC):
                    pav = avps.tile([65, 512], F32, tag="av")
                    prev_av = None
                    for av in emit_unit_blocks(h, q, pav):
                        if prev_av is not None:
                            prev_av()
                        elif pending is not None:
                            flush_pending()
                        prev_av = av
                    after = (
                        [
                            (
                                lambda q=q, ts=ts, oc=oc: outproj_group(
                                    q, ts, oc, use_act=(q == NCH - 1 and oc == 0)
                                )
                            )
                            for ts in range(4)
                            for oc in range(2)
                        ]
                        if h == HPC - 1
                        else []
                    )
                    pending = (
                        prev_av,
                        lambda h=h, q=q, pav=pav: normalize(h, q, pav),
                        after,
                    )
                    for _ in range(2):
                        if pending_outproj:
                            pending_outproj.pop(0)()

            proj_chunk(0)
            proj_chunk(1)
            attn_chunk(0)
            proj_chunk(2)
            attn_chunk(1)
            proj_chunk(3)
            attn_chunk(2)
            attn_chunk(3)
            flush_pending()
            for th in pending_outproj:
                th()

    nc.finalize()
    return nc


def _prep_core_inputs(x, w_qkv, w_out, core):
    b, hg = core // HG, core % HG
    xT = np.ascontiguousarray(x[b].T)
    wq = w_qkv[0:C] * np.float32(1.0 / np.sqrt(D))
    wk = w_qkv[C : 2 * C]
    wv = w_qkv[2 * C : 3 * C]
    h0 = HPC * hg
    rows = []
    for p in range(2):
        rows.append(wq[64 * (h0 + 2 * p) : 64 * (h0 + 2 * p + 2)])
    for p in range(2):
        rows.append(wk[64 * (h0 + 2 * p) : 64 * (h0 + 2 * p + 2)])
    wqk_lhsT = np.ascontiguousarray(
        np.concatenate(rows, axis=0).T.reshape(CK, 128, 512).transpose(1, 0, 2)
    )
    wv_rhsT = np.ascontiguousarray(wv[64 * h0 : 64 * (h0 + HPC)].T)
    wout_pairs = np.ascontiguousarray(
        w_out[:, 64 * h0 : 64 * (h0 + HPC)].T
    ).reshape(2, 128, C)
    # xT and wv are sent p-major shuffled ([128, CK, ...]) so the kernel can
    # load several contraction k-tiles with one contiguous DMA.
    xTs = np.ascontiguousarray(
        xT.reshape(CK, 128, T).transpose(1, 0, 2)
    )
    wvs = np.ascontiguousarray(
        wv_rhsT.reshape(CK, 128, 256).transpose(1, 0, 2)
    )
    return {
        "ones": np.ones((128, 4), dtype=np.float32),
        "xT": xTs.astype(np.float32),
        "wqk": wqk_lhsT.astype(np.float32),
        "wv": wvs.astype(np.float32),
        "wout": wout_pairs.astype(np.float32),
    }


def kernel(x, w_qkv, w_out):
    from concourse.bass_utils import run_bass_kernel_spmd

    global _NC
    x = np.asarray(x, dtype=np.float32)
    w_qkv = np.asarray(w_qkv, dtype=np.float32)
    w_out = np.asarray(w_out, dtype=np.float32)

    in_maps = [_prep_core_inputs(x, w_qkv, w_out, c) for c in range(N_CORES)]
    if _NC is None:
        _NC = _build_nc()
    res = run_bass_kernel_spmd(_NC, in_maps, core_ids=list(range(N_CORES)))
    out = np.zeros((B, T, C), dtype=np.float32)
    for c in range(N_CORES):
        out[c // HG] += res.results[c]["y"]
    return out
